# revision 1
# baseline (speedup 1.0000x reference)
"""Causal attention (B=4,H=16,S=2048,D=64) on 8 NeuronCores via Bass/Tile.

Strategy (per core = 8 heads of the 64 B*H heads):
- Host pre-transposes Q,K to [d, s] layout and assembles one combined
  fp32r tensor per core; V gets a ones-column appended (denominator).
- Device computes S^T[k,q] tiles = K^T.T @ Q^T (contraction d), adds the
  causal-triangle mask on diagonal 128-blocks via a constant rank-128
  matmul (-1e30 entries -> exp underflows to 0), applies exp on ScalarE
  (scale=1/sqrt(64) folded in, no max-subtraction: scores are ~N(0,1)),
  then accumulates out'^T[65, q] = V_aug.T @ E^T over k-tiles in PSUM.
  Fully-masked tiles are skipped (block-causal sparsity).
- acc rows 0-63 = unnormalized out^T, row 64 = softmax denominator.
  Host divides and transposes back. No max-subtract is safe: scores*scale
  are a few sigma of N(0,1); exp stays in fp32 range.
"""
import os
import sys

sys.path.insert(0, "/opt/trn_rl_repo")

import numpy as np

B, H, S, D = 4, 16, 2048, 64
NCORES = 8
HPC = (B * H) // NCORES        # heads per core = 8
NKT = S // 128                 # k-tiles per head = 16
NQB = S // 512                 # q blocks per head = 4
VCOLS = NKT * (D + 1)          # 16*65 = 1040
PAIR_COLS = 2 * S + 2 * VCOLS  # KT[128,2048] QT[128,2048] V_a V_b = 6176
NPAIR = HPC // 2               # 4
SCALE = 1.0 / 8.0              # 1/sqrt(D)
NEG_BIG = -1.0e30

last_exec_time_ns = None

_prog_cache = {}


def _install_trace_hook():
    """Inject antenv.axon_hooks (missing from this image) so trace=True can
    capture NTFF profiles. Degrades silently if anything is unavailable."""
    import types

    try:
        import antenv

        if "antenv.axon_hooks" in sys.modules:
            return
        mod = types.ModuleType("antenv.axon_hooks")
        state = {"hook": None}
        mod.set_axon_ntff_profile_hook = lambda h: state.__setitem__("hook", h)
        mod.get_axon_ntff_profile_hook = lambda: state["hook"]
        sys.modules["antenv.axon_hooks"] = mod
        antenv.axon_hooks = mod
        from trn_agent_boot.trn_boot import _ntff_profile_via_ctypes

        hook = _ntff_profile_via_ctypes("/opt/axon/libaxon_pjrt.so")
        if hook is not None:
            mod.set_axon_ntff_profile_hook(hook)
    except Exception:
        pass


def _build_program():
    import concourse.bass as bass  # noqa: F401
    import concourse.mybir as mybir
    import concourse.tile as tile
    from concourse import bacc

    F32 = mybir.dt.float32
    F32R = mybir.dt.float32r
    BF16 = mybir.dt.bfloat16
    EXP = mybir.ActivationFunctionType.Exp

    nc = bacc.Bacc()
    CMB = nc.declare_dram_parameter(
        "CMB", [128, NPAIR * PAIR_COLS], F32R, isOutput=False
    )
    MSK = nc.declare_dram_parameter("MSK", [128, 256], BF16, isOutput=False)
    OUT = nc.declare_dram_parameter("OUT", [HPC, D + 1, S], F32, isOutput=True)

    with tile.TileContext(nc) as tc:
        with (
            tc.tile_pool(name="cmbp", bufs=2) as cmbp,
            tc.tile_pool(name="singles", bufs=1) as singles,
            tc.tile_pool(name="etp", bufs=4) as etp,
            tc.tile_pool(name="stp", bufs=2, space="PSUM") as stp,
            tc.tile_pool(name="accp", bufs=1, space="PSUM") as accp,
        ):
            msk = singles.tile([128, 256], BF16, tag="msk")
            nc.sync.dma_start(out=msk, in_=MSK[:])
            mska = msk[:, 0:128]
            mskb = msk[:, 128:256]

            for pair in range(NPAIR):
                cmb = cmbp.tile(
                    [128, PAIR_COLS], F32R, tag="cmb", name=f"cmb{pair}"
                )
                nc.sync.dma_start(
                    out=cmb,
                    in_=CMB[:, pair * PAIR_COLS:(pair + 1) * PAIR_COLS],
                )
                for sub in range(2):
                    head = 2 * pair + sub
                    base = 64 * sub
                    kt = cmb[base:base + 64, 0:S]
                    qt = cmb[base:base + 64, S:2 * S]
                    va = cmb[:, 2 * S + sub * VCOLS: 2 * S + (sub + 1) * VCOLS]

                    accs = [
                        accp.tile([D + 1, 512], F32, tag=f"acc{qj}",
                                  name=f"acc_h{head}_q{qj}")
                        for qj in range(NQB)
                    ]

                    for ki in range(NKT):
                        sg = 128 * ki          # first allowed q col
                        lhs_k = kt[:, 128 * ki:128 * (ki + 1)]
                        va_k = va.rearrange(
                            "p (t c) -> p t c", t=NKT
                        )[:, ki, :]            # [128, 65]

                        for c in range(2):     # 1024-wide chunks
                            c_lo, c_hi = 1024 * c, 1024 * (c + 1)
                            if sg >= c_hi:
                                continue       # chunk fully masked
                            s = max(0, sg - c_lo)  # within-chunk start
                            st = stp.tile(
                                [128, 1024], F32, tag="st",
                                name=f"st_h{head}_k{ki}_c{c}",
                            )
                            # ---- S^T matmuls (N<=512, one PSUM bank each)
                            if s < 512:
                                nc.tensor.matmul(
                                    st[:, s:512], lhs_k,
                                    qt[:, c_lo + s:c_lo + 512],
                                    start=True, stop=True,
                                )
                                nc.tensor.matmul(
                                    st[:, 512:1024], lhs_k,
                                    qt[:, c_lo + 512:c_hi],
                                    start=True, stop=True,
                                )
                            else:
                                nc.tensor.matmul(
                                    st[:, s:1024], lhs_k,
                                    qt[:, c_lo + s:c_hi],
                                    start=True, stop=True,
                                )
                            # ---- causal triangle on the diagonal block
                            if c_lo <= sg < c_hi:
                                dlo = s
                                if dlo % 512 + 128 <= 512:
                                    nc.tensor.matmul(
                                        st[:, dlo:dlo + 128], mska, mskb,
                                        start=False, stop=True,
                                    )
                                else:  # crosses PSUM bank: split 64/64
                                    nc.tensor.matmul(
                                        st[:, dlo:dlo + 64], mska,
                                        mskb[:, 0:64],
                                        start=False, stop=True,
                                    )
                                    nc.tensor.matmul(
                                        st[:, dlo + 64:dlo + 128], mska,
                                        mskb[:, 64:128],
                                        start=False, stop=True,
                                    )
                            # ---- exp (wide ACT op over both banks)
                            et = etp.tile(
                                [128, 1024], F32R, tag="et",
                                name=f"et_h{head}_k{ki}_c{c}",
                            )
                            nc.scalar.activation(
                                et[:, s:1024], st[:, s:1024], EXP, scale=SCALE
                            )
                            # ---- PV accumulation (per 512-half)
                            for hh in range(2):
                                h_lo = 512 * hh
                                if s >= h_lo + 512:
                                    continue
                                p_lo = max(s, h_lo)
                                qj = 2 * c + hh
                                nc.tensor.matmul(
                                    accs[qj][:, p_lo - h_lo:512],
                                    va_k,
                                    et[:, p_lo:h_lo + 512],
                                    start=(ki == 0),
                                    stop=(ki == 4 * qj + 3),
                                )
                    # ---- write out accumulators (PSUM -> SBUF -> DRAM)
                    for qj in range(NQB):
                        ob = etp.tile(
                            [D + 1, 512], mybir.dt.float32, tag="ob",
                            name=f"ob_h{head}_q{qj}",
                        )
                        nc.vector.tensor_copy(ob, accs[qj])
                        nc.sync.dma_start(
                            out=OUT[head, :, 512 * qj:512 * (qj + 1)],
                            in_=ob,
                        )
    nc.finalize()
    return nc


def _get_program():
    if "nc" not in _prog_cache:
        _prog_cache["nc"] = _build_program()
    return _prog_cache["nc"]


def _mask_matrices():
    # M = A.T @ B adds NEG_BIG to entries (r, c) with c < r of a 128x128
    # block: A[j, r] = 1 if r > j (j<127); B[j, j] = NEG_BIG.
    import ml_dtypes

    A = (np.arange(128)[None, :] > np.arange(128)[:, None]).astype(np.float32)
    A[127, :] = 0.0
    Bm = np.zeros((128, 128), dtype=np.float32)
    idx = np.arange(127)
    Bm[idx, idx] = NEG_BIG
    return np.concatenate([A, Bm], axis=1).astype(ml_dtypes.bfloat16)


def kernel(q, k, v, mask):
    global last_exec_time_ns
    q = np.asarray(q, dtype=np.float32)
    k = np.asarray(k, dtype=np.float32)
    v = np.asarray(v, dtype=np.float32)
    mask = np.asarray(mask).astype(bool)

    # This kernel specializes the causal (lower-triangular) mask from the
    # module; for any other mask fall back to a host reference.
    tril = np.tril(np.ones((S, S), dtype=bool))
    if mask.shape != (1, 1, S, S) or not np.array_equal(mask[0, 0], tril):
        scores = np.einsum("bhqd,bhkd->bhqk", q, k) / np.sqrt(np.float32(D))
        scores = np.where(mask, scores, -np.inf)
        m = scores.max(-1, keepdims=True)
        e = np.exp(scores - m)
        return (np.einsum("bhqk,bhkd->bhqd", e / e.sum(-1, keepdims=True), v)
                .astype(np.float32))

    _install_trace_hook()
    from concourse.bass_utils import run_bass_kernel_spmd

    nc = _get_program()

    qf = q.reshape(B * H, S, D)
    kf = k.reshape(B * H, S, D)
    vf = v.reshape(B * H, S, D)

    msk_np = _mask_matrices()
    in_maps = []
    for core in range(NCORES):
        pairs = []
        for p in range(NPAIR):
            hA = core * HPC + 2 * p
            hB = hA + 1
            ktp = np.concatenate(
                [kf[hA].T, kf[hB].T], axis=0
            )  # [128, 2048]
            qtp = np.concatenate([qf[hA].T, qf[hB].T], axis=0)
            vas = []
            for h in (hA, hB):
                vt = vf[h].reshape(NKT, 128, D).transpose(1, 0, 2)  # [128,16,64]
                va = np.concatenate(
                    [vt, np.ones((128, NKT, 1), dtype=np.float32)], axis=2
                ).reshape(128, VCOLS)
                vas.append(va)
            pairs.append(np.concatenate([ktp, qtp, vas[0], vas[1]], axis=1))
        cmb = np.ascontiguousarray(np.concatenate(pairs, axis=1))
        in_maps.append({"CMB": cmb, "MSK": msk_np})

    trace = bool(os.environ.get("ATTN_TRACE"))
    res = run_bass_kernel_spmd(
        nc, in_maps, list(range(NCORES)), trace=trace
    )
    last_exec_time_ns = res.exec_time_ns

    out = np.empty((B * H, S, D), dtype=np.float32)
    for core in range(NCORES):
        acc = res.results[core]["OUT"]  # [HPC, 65, S]
        o = acc[:, :D, :] / acc[:, D:D + 1, :]
        out[core * HPC:(core + 1) * HPC] = o.transpose(0, 2, 1)
    return out.reshape(B, H, S, D)



# revision 2
# speedup vs baseline: 1.4812x; 1.4812x over previous
"""Causal attention (B=4,H=16,S=2048,D=64) on 8 NeuronCores via Bass/Tile.

v2 strategy (per core = 8 heads of the 64 B*H heads):
- All matmul inputs in fp16 (1 cyc/col on the PE array vs ~3-4 for fp32r).
- Layout per head: K^T/Q^T [64, S] fp16; V augmented with a ones column
  (denominator) as [128, 16, 65] fp16.
- Loop: 2 q-blocks of 1024 per head; per (qb, ki) a [128, <=1024] score
  tile S^T = K_tile.T @ Q in PSUM; exp is computed either exactly on the
  Scalar engine (ACT, with scale=1/8 folded) or via a one-instruction
  Schraudolph fast-exp on the Vector engine (tensor_scalar writing the
  fp16 bit pattern through an int16 output); tiles covering q<256 are
  forced to the exact path, the rest are greedily load-balanced.
- The causal triangle on diagonal 128-blocks is applied by multiplying
  the exp tile with a 0/1 triangle on GpSimd (no -inf mask matmuls).
- PV: acc[65, 1024] += V_aug.T @ E per k-tile, accumulated in PSUM.
- Accumulators are copied to SBUF fp16 (alternating ACT/DVE) and DMA'd
  out; host divides by the denominator row and transposes back.
- No max-subtraction: scores*scale ~ N(0,1), exp stays in range.
"""
import os
import sys

sys.path.insert(0, "/opt/trn_rl_repo")

import numpy as np

B, H, S, D = 4, 16, 2048, 64
NCORES = 8
HPC = (B * H) // NCORES        # heads per core = 8
NKT = S // 128                 # 128-wide k-tiles per head = 16
QB = 1024                      # q-block width
NQB = S // QB                  # q blocks per head = 2
VCOLS = NKT * (D + 1)          # 16*65 = 1040
PAIR_COLS = 2 * S + 2 * VCOLS  # KT[128,2048] QT[128,2048] V_a V_b = 6176
NPAIR = HPC // 2               # 4
SCALE = 1.0 / 8.0              # 1/sqrt(D)

# Schraudolph fast-exp onto the fp16 bit pattern:
#   i16 = round(raw_score * FE_A + FE_B);  bitcast(i16) ~= exp(raw*SCALE)
# max rel err ~3.0%; only used for tiles whose q-columns are all >= 256
# (rows there average >=257 softmax terms, so the sawtooth error cancels).
FE_A = float(1024.0 * np.log2(np.e) * SCALE)
FE_B = float(15360.0 - 44.7)

last_exec_time_ns = None

_prog_cache = {}


def _install_trace_hook():
    """Inject antenv.axon_hooks (missing from this image) so trace=True can
    capture NTFF profiles. Degrades silently if anything is unavailable."""
    import types

    try:
        import antenv

        if "antenv.axon_hooks" in sys.modules:
            return
        mod = types.ModuleType("antenv.axon_hooks")
        state = {"hook": None}
        mod.set_axon_ntff_profile_hook = lambda h: state.__setitem__("hook", h)
        mod.get_axon_ntff_profile_hook = lambda: state["hook"]
        sys.modules["antenv.axon_hooks"] = mod
        antenv.axon_hooks = mod
        from trn_agent_boot.trn_boot import _ntff_profile_via_ctypes

        hook = _ntff_profile_via_ctypes("/opt/axon/libaxon_pjrt.so")
        if hook is not None:
            mod.set_axon_ntff_profile_hook(hook)
    except Exception:
        pass


def _plan_exp_engines():
    """Static per-(head, qb, ki) assignment of the exp op to ACT or DVE.

    Returns dict (head, qb, ki) -> 'act' | 'dve'. Tiles containing
    q-columns < 256 must be exact (ACT); everything else goes to
    whichever engine has the lower modeled finish time.
    """
    ACT_NS_COL, ACT_NS_FIX = 1.01, 295.0
    DVE_NS_COL, DVE_NS_FIX = 1.04, 160.0
    # init with the out-copy work each engine does (split evenly):
    load = {"act": 0.0, "dve": 0.0}
    n_copies = HPC * NQB
    load["act"] += (n_copies / 2) * (QB * ACT_NS_COL + ACT_NS_FIX)
    load["dve"] += (n_copies / 2) * (QB * DVE_NS_COL + DVE_NS_FIX)
    plan = {}
    for head in range(HPC):
        for qb in range(NQB):
            q0 = QB * qb
            kmax = (q0 + QB) // 128
            for ki in range(kmax):
                sg = max(0, 128 * ki - q0)
                cols = QB - sg
                forced = q0 + sg < 256
                ca = cols * ACT_NS_COL + ACT_NS_FIX
                cd = cols * DVE_NS_COL + DVE_NS_FIX
                if forced or load["act"] + ca <= load["dve"] + cd:
                    plan[(head, qb, ki)] = "act"
                    load["act"] += ca
                else:
                    plan[(head, qb, ki)] = "dve"
                    load["dve"] += cd
    return plan


def _build_program():
    import concourse.bass as bass  # noqa: F401
    import concourse.mybir as mybir
    import concourse.tile as tile
    from concourse import bacc

    F16 = mybir.dt.float16
    F32 = mybir.dt.float32
    I16 = mybir.dt.int16
    EXP = mybir.ActivationFunctionType.Exp
    MULT = mybir.AluOpType.mult
    ADD = mybir.AluOpType.add

    plan = _plan_exp_engines()

    nc = bacc.Bacc()
    CMB = nc.declare_dram_parameter(
        "CMB", [128, NPAIR * PAIR_COLS], F16, isOutput=False
    )
    TRI = nc.declare_dram_parameter("TRI", [128, 128], F16, isOutput=False)
    OUT = nc.declare_dram_parameter("OUT", [HPC, D + 1, S], F16, isOutput=True)

    with tile.TileContext(nc) as tc:
        with (
            tc.tile_pool(name="cmbp", bufs=2) as cmbp,
            tc.tile_pool(name="singles", bufs=1) as singles,
            tc.tile_pool(name="etp", bufs=3) as etp,
            tc.tile_pool(name="obp", bufs=2) as obp,
            tc.tile_pool(name="stp", bufs=2, space="PSUM") as stp,
            tc.tile_pool(name="accp", bufs=2, space="PSUM") as accp,
        ):
            tri = singles.tile([128, 128], F16, tag="tri")
            nc.sync.dma_start(out=tri, in_=TRI[:])

            copy_tick = 0
            for pair in range(NPAIR):
                cmb = cmbp.tile(
                    [128, PAIR_COLS], F16, tag="cmb", name=f"cmb{pair}"
                )
                nc.sync.dma_start(
                    out=cmb,
                    in_=CMB[:, pair * PAIR_COLS:(pair + 1) * PAIR_COLS],
                )
                for sub in range(2):
                    head = 2 * pair + sub
                    base = 64 * sub
                    kt = cmb[base:base + 64, 0:S]
                    qt = cmb[base:base + 64, S:2 * S]
                    va = cmb[:, 2 * S + sub * VCOLS: 2 * S + (sub + 1) * VCOLS]
                    va_r = va.rearrange("p (t c) -> p t c", t=NKT)

                    for qb in range(NQB):
                        q0 = QB * qb
                        kmax = (q0 + QB) // 128
                        acc = accp.tile(
                            [D + 1, QB], F32, tag="acc",
                            name=f"acc_h{head}_qb{qb}",
                        )
                        ets = {}

                        def do_st(ki):
                            """S^T matmuls for k-tile ki into a fresh PSUM
                            tile, then exp (+ diag mask) into SBUF fp16."""
                            sg = max(0, 128 * ki - q0)
                            st = stp.tile(
                                [128, QB], F32, tag="st",
                                name=f"st_h{head}_qb{qb}_k{ki}",
                            )
                            lhs_k = kt[:, 128 * ki:128 * (ki + 1)]
                            c0 = sg
                            while c0 < QB:
                                c1 = min(QB, (c0 // 512 + 1) * 512)
                                nc.tensor.matmul(
                                    st[:, c0:c1], lhs_k,
                                    qt[:, q0 + c0:q0 + c1],
                                    start=True, stop=True,
                                )
                                c0 = c1
                            et = etp.tile(
                                [128, QB], F16, tag="et",
                                name=f"et_h{head}_qb{qb}_k{ki}",
                            )
                            if plan[(head, qb, ki)] == "act":
                                nc.scalar.activation(
                                    et[:, sg:QB], st[:, sg:QB], EXP,
                                    scale=SCALE,
                                )
                            else:
                                nc.vector.tensor_scalar(
                                    et[:, sg:QB].bitcast(I16),
                                    st[:, sg:QB],
                                    FE_A, FE_B, MULT, ADD,
                                )
                            if 128 * ki >= q0:  # diagonal block in this tile
                                nc.gpsimd.tensor_tensor(
                                    et[:, sg:sg + 128],
                                    et[:, sg:sg + 128],
                                    tri[:, :],
                                    MULT,
                                )
                            ets[ki] = (et, sg)

                        def do_pv(ki):
                            et, sg = ets.pop(ki)
                            va_k = va_r[:, ki, :]  # [128, 65]
                            c0 = sg
                            while c0 < QB:
                                c1 = min(QB, (c0 // 512 + 1) * 512)
                                last_ki = (q0 + c1 - 1) // 128
                                nc.tensor.matmul(
                                    acc[:, c0:c1], va_k, et[:, c0:c1],
                                    start=(ki == 0),
                                    stop=(ki == last_ki),
                                )
                                c0 = c1

                        # software-pipelined issue: S(k0),S(k1),PV(k0),...
                        do_st(0)
                        for ki in range(1, kmax):
                            do_st(ki)
                            do_pv(ki - 1)
                        do_pv(kmax - 1)

                        ob = obp.tile(
                            [D + 1, QB], F16, tag="ob",
                            name=f"ob_h{head}_qb{qb}",
                        )
                        if copy_tick % 2 == 0:
                            nc.scalar.copy(ob, acc)
                        else:
                            nc.vector.tensor_copy(ob, acc)
                        copy_tick += 1
                        nc.sync.dma_start(
                            out=OUT[head, :, q0:q0 + QB], in_=ob,
                        )
    nc.finalize()
    return nc


def _get_program():
    if "nc" not in _prog_cache:
        _prog_cache["nc"] = _build_program()
    return _prog_cache["nc"]


def kernel(q, k, v, mask):
    global last_exec_time_ns
    q = np.asarray(q, dtype=np.float32)
    k = np.asarray(k, dtype=np.float32)
    v = np.asarray(v, dtype=np.float32)
    mask = np.asarray(mask).astype(bool)

    # This kernel specializes the causal (lower-triangular) mask from the
    # module; for any other mask fall back to a host reference.
    tril = np.tril(np.ones((S, S), dtype=bool))
    if mask.shape != (1, 1, S, S) or not np.array_equal(mask[0, 0], tril):
        scores = np.einsum("bhqd,bhkd->bhqk", q, k) / np.sqrt(np.float32(D))
        scores = np.where(mask, scores, -np.inf)
        m = scores.max(-1, keepdims=True)
        e = np.exp(scores - m)
        return (np.einsum("bhqk,bhkd->bhqd", e / e.sum(-1, keepdims=True), v)
                .astype(np.float32))

    _install_trace_hook()
    from concourse.bass_utils import run_bass_kernel_spmd

    nc = _get_program()

    qf = q.reshape(B * H, S, D).astype(np.float16)
    kf = k.reshape(B * H, S, D).astype(np.float16)
    vf = v.reshape(B * H, S, D).astype(np.float16)

    tri_np = np.triu(np.ones((128, 128), dtype=np.float16))  # keep k<=q

    in_maps = []
    for core in range(NCORES):
        pairs = []
        for p in range(NPAIR):
            hA = core * HPC + 2 * p
            hB = hA + 1
            ktp = np.concatenate(
                [kf[hA].T, kf[hB].T], axis=0
            )  # [128, 2048]
            qtp = np.concatenate([qf[hA].T, qf[hB].T], axis=0)
            vas = []
            for h in (hA, hB):
                vt = vf[h].reshape(NKT, 128, D).transpose(1, 0, 2)
                va = np.concatenate(
                    [vt, np.ones((128, NKT, 1), dtype=np.float16)], axis=2
                ).reshape(128, VCOLS)
                vas.append(va)
            pairs.append(np.concatenate([ktp, qtp, vas[0], vas[1]], axis=1))
        cmb = np.ascontiguousarray(np.concatenate(pairs, axis=1))
        in_maps.append({"CMB": cmb, "TRI": tri_np})

    trace = bool(os.environ.get("ATTN_TRACE"))
    res = run_bass_kernel_spmd(
        nc, in_maps, list(range(NCORES)), trace=trace
    )
    last_exec_time_ns = res.exec_time_ns

    out = np.empty((B * H, S, D), dtype=np.float32)
    for core in range(NCORES):
        acc = res.results[core]["OUT"].astype(np.float32)  # [HPC, 65, S]
        o = acc[:, :D, :] / acc[:, D:D + 1, :]
        out[core * HPC:(core + 1) * HPC] = o.transpose(0, 2, 1)
    return out.reshape(B, H, S, D)


# revision 5
# speedup vs baseline: 1.5196x; 1.0259x over previous
"""Causal attention (B=4,H=16,S=2048,D=64) on 8 NeuronCores via Bass/Tile.

v2 strategy (per core = 8 heads of the 64 B*H heads):
- All matmul inputs in fp16 (1 cyc/col on the PE array vs ~3-4 for fp32r).
- Layout per head: K^T/Q^T [64, S] fp16; V augmented with a ones column
  (denominator) as [128, 16, 65] fp16.
- Loop: 2 q-blocks of 1024 per head; per (qb, ki) a [128, <=1024] score
  tile S^T = K_tile.T @ Q in PSUM; exp is computed either exactly on the
  Scalar engine (ACT, with scale=1/8 folded) or via a one-instruction
  Schraudolph fast-exp on the Vector engine (tensor_scalar writing the
  fp16 bit pattern through an int16 output); tiles covering q<256 are
  forced to the exact path, the rest are greedily load-balanced.
- The causal triangle on diagonal 128-blocks is applied by multiplying
  the exp tile with a 0/1 triangle on GpSimd (no -inf mask matmuls).
- PV: acc[65, 1024] += V_aug.T @ E per k-tile, accumulated in PSUM.
- Accumulators are copied to SBUF fp16 (alternating ACT/DVE) and DMA'd
  out; host divides by the denominator row and transposes back.
- No max-subtraction: scores*scale ~ N(0,1), exp stays in range.
"""
import os
import sys

sys.path.insert(0, "/opt/trn_rl_repo")

import numpy as np

B, H, S, D = 4, 16, 2048, 64
NCORES = 8
HPC = (B * H) // NCORES        # heads per core = 8
NKT = S // 128                 # 128-wide k-tiles per head = 16
QB = 1024                      # q-block width
NQB = S // QB                  # q blocks per head = 2
VCOLS = NKT * (D + 1)          # 16*65 = 1040
PAIR_COLS = 2 * S + 2 * VCOLS  # KT[128,2048] QT[128,2048] V_a V_b = 6176
NPAIR = HPC // 2               # 4
SCALE = 1.0 / 8.0              # 1/sqrt(D)

# Schraudolph fast-exp onto the fp16 bit pattern:
#   i16 = round(raw_score * FE_A + FE_B);  bitcast(i16) ~= exp(raw*SCALE)
# max rel err ~3.0%; only used for tiles whose q-columns are all >= 256
# (rows there average >=257 softmax terms, so the sawtooth error cancels).
FE_A = float(1024.0 * np.log2(np.e) * SCALE)
FE_B = float(15360.0 - 44.7)

last_exec_time_ns = None

_prog_cache = {}


def _install_trace_hook():
    """Inject antenv.axon_hooks (missing from this image) so trace=True can
    capture NTFF profiles. Degrades silently if anything is unavailable."""
    import types

    try:
        import antenv

        if "antenv.axon_hooks" in sys.modules:
            return
        mod = types.ModuleType("antenv.axon_hooks")
        state = {"hook": None}
        mod.set_axon_ntff_profile_hook = lambda h: state.__setitem__("hook", h)
        mod.get_axon_ntff_profile_hook = lambda: state["hook"]
        sys.modules["antenv.axon_hooks"] = mod
        antenv.axon_hooks = mod
        from trn_agent_boot.trn_boot import _ntff_profile_via_ctypes

        hook = _ntff_profile_via_ctypes("/opt/axon/libaxon_pjrt.so")
        if hook is not None:
            mod.set_axon_ntff_profile_hook(hook)
    except Exception:
        pass


def _plan_exp_engines():
    """Static per-(head, qb, ki) assignment of the exp op to ACT or DVE.

    Returns dict (head, qb, ki) -> 'act' | 'dve'. Tiles containing
    q-columns < 256 must be exact (ACT); everything else goes to
    whichever engine has the lower modeled finish time.
    """
    ACT_NS_COL, ACT_NS_FIX = 1.01, 295.0
    DVE_NS_COL, DVE_NS_FIX = 1.04, 160.0
    # init with the out-copy work each engine does (split evenly):
    load = {"act": 0.0, "dve": 0.0}
    n_copies = HPC * NQB
    load["act"] += (n_copies / 2) * (QB * ACT_NS_COL + ACT_NS_FIX)
    load["dve"] += (n_copies / 2) * (QB * DVE_NS_COL + DVE_NS_FIX)
    plan = {}
    for head in range(HPC):
        for qb in range(NQB):
            q0 = QB * qb
            kmax = (q0 + QB) // 128
            for ki in range(kmax):
                sg = max(0, 128 * ki - q0)
                cols = QB - sg
                forced = q0 + sg < 256
                ca = cols * ACT_NS_COL + ACT_NS_FIX
                cd = cols * DVE_NS_COL + DVE_NS_FIX
                if forced or load["act"] + ca <= load["dve"] + cd:
                    plan[(head, qb, ki)] = "act"
                    load["act"] += ca
                else:
                    plan[(head, qb, ki)] = "dve"
                    load["dve"] += cd
    return plan


def _build_program():
    import concourse.bass as bass  # noqa: F401
    import concourse.mybir as mybir
    import concourse.tile as tile
    from concourse import bacc

    F16 = mybir.dt.float16
    F32 = mybir.dt.float32
    I16 = mybir.dt.int16
    EXP = mybir.ActivationFunctionType.Exp
    MULT = mybir.AluOpType.mult
    ADD = mybir.AluOpType.add

    plan = _plan_exp_engines()

    nc = bacc.Bacc()
    CMB = nc.declare_dram_parameter(
        "CMB", [128, NPAIR * PAIR_COLS], F16, isOutput=False
    )
    TRI = nc.declare_dram_parameter("TRI", [128, 640], F16, isOutput=False)
    OUT = nc.declare_dram_parameter("OUT", [HPC, D + 1, S], F16, isOutput=True)

    with tile.TileContext(nc) as tc:
        with (
            tc.tile_pool(name="cmbp", bufs=2) as cmbp,
            tc.tile_pool(name="singles", bufs=1) as singles,
            tc.tile_pool(name="etp", bufs=3) as etp,
            tc.tile_pool(name="obp", bufs=2) as obp,
            tc.tile_pool(name="stp", bufs=2, space="PSUM") as stp,
            tc.tile_pool(name="accp", bufs=2, space="PSUM") as accp,
        ):
            trib = singles.tile([128, 640], F16, tag="tri")
            nc.sync.dma_start(out=trib, in_=TRI[:])
            tri = trib[:, 0:128]

            # PE warm-up: the HAM clock gate only un-throttles (1.2 ->
            # 2.4 GHz) under sustained full-height (128-row) matmul
            # activity, and this kernel's 64-contraction score matmuls
            # never trigger it on their own. ~30 chained 512-col matmuls
            # warm the array while the first CMB DMA is in flight; once
            # warm, the dense matmul stream keeps it warm.
            for wi in range(30):
                wt = stp.tile([128, QB], F32, tag="st", name=f"warm{wi}")
                nc.tensor.matmul(
                    wt[:, 0:512], trib[:, 0:128], trib[:, 128:640],
                    start=True, stop=True,
                )

            copy_tick = 0
            for pair in range(NPAIR):
                cmb = cmbp.tile(
                    [128, PAIR_COLS], F16, tag="cmb", name=f"cmb{pair}"
                )
                nc.sync.dma_start(
                    out=cmb,
                    in_=CMB[:, pair * PAIR_COLS:(pair + 1) * PAIR_COLS],
                )
                for sub in range(2):
                    head = 2 * pair + sub
                    base = 64 * sub
                    kt = cmb[base:base + 64, 0:S]
                    qt = cmb[base:base + 64, S:2 * S]
                    va = cmb[:, 2 * S + sub * VCOLS: 2 * S + (sub + 1) * VCOLS]
                    va_r = va.rearrange("p (t c) -> p t c", t=NKT)

                    for qb in range(NQB):
                        q0 = QB * qb
                        kmax = (q0 + QB) // 128
                        acc = accp.tile(
                            [D + 1, QB], F32, tag="acc",
                            name=f"acc_h{head}_qb{qb}",
                        )
                        ets = {}

                        def do_st(ki):
                            """S^T matmuls for k-tile ki into a fresh PSUM
                            tile, then exp (+ diag mask) into SBUF fp16."""
                            sg = max(0, 128 * ki - q0)
                            st = stp.tile(
                                [128, QB], F32, tag="st",
                                name=f"st_h{head}_qb{qb}_k{ki}",
                            )
                            lhs_k = kt[:, 128 * ki:128 * (ki + 1)]
                            c0 = sg
                            while c0 < QB:
                                c1 = min(QB, (c0 // 512 + 1) * 512)
                                nc.tensor.matmul(
                                    st[:, c0:c1], lhs_k,
                                    qt[:, q0 + c0:q0 + c1],
                                    start=True, stop=True,
                                )
                                c0 = c1
                            et = etp.tile(
                                [128, QB], F16, tag="et",
                                name=f"et_h{head}_qb{qb}_k{ki}",
                            )
                            if plan[(head, qb, ki)] == "act":
                                nc.scalar.activation(
                                    et[:, sg:QB], st[:, sg:QB], EXP,
                                    scale=SCALE,
                                )
                            else:
                                nc.vector.tensor_scalar(
                                    et[:, sg:QB].bitcast(I16),
                                    st[:, sg:QB],
                                    FE_A, FE_B, MULT, ADD,
                                )
                            if 128 * ki >= q0:  # diagonal block in this tile
                                nc.gpsimd.tensor_tensor(
                                    et[:, sg:sg + 128],
                                    et[:, sg:sg + 128],
                                    tri[:, :],
                                    MULT,
                                )
                            ets[ki] = (et, sg)

                        def do_pv(ki):
                            et, sg = ets.pop(ki)
                            va_k = va_r[:, ki, :]  # [128, 65]
                            c0 = sg
                            while c0 < QB:
                                c1 = min(QB, (c0 // 512 + 1) * 512)
                                last_ki = (q0 + c1 - 1) // 128
                                nc.tensor.matmul(
                                    acc[:, c0:c1], va_k, et[:, c0:c1],
                                    start=(ki == 0),
                                    stop=(ki == last_ki),
                                )
                                c0 = c1

                        # software-pipelined issue: S(k0),S(k1),PV(k0),...
                        do_st(0)
                        for ki in range(1, kmax):
                            do_st(ki)
                            do_pv(ki - 1)
                        do_pv(kmax - 1)

                        ob = obp.tile(
                            [D + 1, QB], F16, tag="ob",
                            name=f"ob_h{head}_qb{qb}",
                        )
                        if copy_tick % 2 == 0:
                            nc.scalar.copy(ob, acc)
                        else:
                            nc.vector.tensor_copy(ob, acc)
                        copy_tick += 1
                        nc.sync.dma_start(
                            out=OUT[head, :, q0:q0 + QB], in_=ob,
                        )
    nc.finalize()
    return nc


def _get_program():
    if "nc" not in _prog_cache:
        _prog_cache["nc"] = _build_program()
    return _prog_cache["nc"]


def kernel(q, k, v, mask):
    global last_exec_time_ns
    q = np.asarray(q, dtype=np.float32)
    k = np.asarray(k, dtype=np.float32)
    v = np.asarray(v, dtype=np.float32)
    mask = np.asarray(mask).astype(bool)

    # This kernel specializes the causal (lower-triangular) mask from the
    # module; for any other mask fall back to a host reference.
    tril = np.tril(np.ones((S, S), dtype=bool))
    if mask.shape != (1, 1, S, S) or not np.array_equal(mask[0, 0], tril):
        scores = np.einsum("bhqd,bhkd->bhqk", q, k) / np.sqrt(np.float32(D))
        scores = np.where(mask, scores, -np.inf)
        m = scores.max(-1, keepdims=True)
        e = np.exp(scores - m)
        return (np.einsum("bhqk,bhkd->bhqd", e / e.sum(-1, keepdims=True), v)
                .astype(np.float32))

    _install_trace_hook()
    from concourse.bass_utils import run_bass_kernel_spmd

    nc = _get_program()

    qf = q.reshape(B * H, S, D).astype(np.float16)
    kf = k.reshape(B * H, S, D).astype(np.float16)
    vf = v.reshape(B * H, S, D).astype(np.float16)

    tri_np = np.ones((128, 640), dtype=np.float16)
    tri_np[:, 0:128] = np.triu(np.ones((128, 128), dtype=np.float16))  # k<=q

    in_maps = []
    for core in range(NCORES):
        pairs = []
        for p in range(NPAIR):
            hA = core * HPC + 2 * p
            hB = hA + 1
            ktp = np.concatenate(
                [kf[hA].T, kf[hB].T], axis=0
            )  # [128, 2048]
            qtp = np.concatenate([qf[hA].T, qf[hB].T], axis=0)
            vas = []
            for h in (hA, hB):
                vt = vf[h].reshape(NKT, 128, D).transpose(1, 0, 2)
                va = np.concatenate(
                    [vt, np.ones((128, NKT, 1), dtype=np.float16)], axis=2
                ).reshape(128, VCOLS)
                vas.append(va)
            pairs.append(np.concatenate([ktp, qtp, vas[0], vas[1]], axis=1))
        cmb = np.ascontiguousarray(np.concatenate(pairs, axis=1))
        in_maps.append({"CMB": cmb, "TRI": tri_np})

    trace = bool(os.environ.get("ATTN_TRACE"))
    res = run_bass_kernel_spmd(
        nc, in_maps, list(range(NCORES)), trace=trace
    )
    last_exec_time_ns = res.exec_time_ns

    out = np.empty((B * H, S, D), dtype=np.float32)
    for core in range(NCORES):
        acc = res.results[core]["OUT"].astype(np.float32)  # [HPC, 65, S]
        o = acc[:, :D, :] / acc[:, D:D + 1, :]
        out[core * HPC:(core + 1) * HPC] = o.transpose(0, 2, 1)
    return out.reshape(B, H, S, D)


# revision 8
# speedup vs baseline: 2.0827x; 1.3706x over previous
"""Causal attention (B=4,H=16,S=2048,D=64) on 8 NeuronCores via Bass/Tile.

v2 strategy (per core = 8 heads of the 64 B*H heads):
- All matmul inputs in fp16 (1 cyc/col on the PE array vs ~3-4 for fp32r).
- Layout per head: K^T/Q^T [64, S] fp16; V augmented with a ones column
  (denominator) as [128, 16, 65] fp16.
- Loop: 2 q-blocks of 1024 per head; per (qb, ki) a [128, <=1024] score
  tile S^T = K_tile.T @ Q in PSUM; exp is computed either exactly on the
  Scalar engine (ACT, with scale=1/8 folded) or via a one-instruction
  Schraudolph fast-exp on the Vector engine (tensor_scalar writing the
  fp16 bit pattern through an int16 output); tiles covering q<256 are
  forced to the exact path, the rest are greedily load-balanced.
- The causal triangle on diagonal 128-blocks is applied by multiplying
  the exp tile with a 0/1 triangle on GpSimd (no -inf mask matmuls).
- PV: acc[65, 1024] += V_aug.T @ E per k-tile, accumulated in PSUM.
- Accumulators are copied to SBUF fp16 (alternating ACT/DVE) and DMA'd
  out; host divides by the denominator row and transposes back.
- No max-subtraction: scores*scale ~ N(0,1), exp stays in range.
"""
import os
import sys

sys.path.insert(0, "/opt/trn_rl_repo")

import numpy as np

B, H, S, D = 4, 16, 2048, 64
NCORES = 8
HPC = (B * H) // NCORES        # heads per core = 8
NKT = S // 128                 # 128-wide k-tiles per head = 16
QB = 1024                      # q-block width
NQB = S // QB                  # q blocks per head = 2
VCOLS = NKT * (D + 1)          # 16*65 = 1040
PAIR_COLS = 3 * S + 2 * VCOLS  # KT | QTA | QTB | V_a | V_b = 8224
NPAIR = HPC // 2               # 4
SCALE = 1.0 / 8.0              # 1/sqrt(D)

# Schraudolph fast-exp onto the fp16 bit pattern:
#   i16 = round(raw_score * FE_A + FE_B);  bitcast(i16) ~= exp(raw*SCALE)
# max rel err ~3.0%; only used for tiles whose q-columns are all >= 256
# (rows there average >=257 softmax terms, so the sawtooth error cancels).
FE_A = float(1024.0 * np.log2(np.e) * SCALE)
FE_B = float(15360.0 - 44.7)

last_exec_time_ns = None

_prog_cache = {}


def _install_trace_hook():
    """Inject antenv.axon_hooks (missing from this image) so trace=True can
    capture NTFF profiles. Degrades silently if anything is unavailable."""
    import types

    try:
        import antenv

        if "antenv.axon_hooks" in sys.modules:
            return
        mod = types.ModuleType("antenv.axon_hooks")
        state = {"hook": None}
        mod.set_axon_ntff_profile_hook = lambda h: state.__setitem__("hook", h)
        mod.get_axon_ntff_profile_hook = lambda: state["hook"]
        sys.modules["antenv.axon_hooks"] = mod
        antenv.axon_hooks = mod
        from trn_agent_boot.trn_boot import _ntff_profile_via_ctypes

        hook = _ntff_profile_via_ctypes("/opt/axon/libaxon_pjrt.so")
        if hook is not None:
            mod.set_axon_ntff_profile_hook(hook)
    except Exception:
        pass


def _plan_exp_engines():
    """Static per-(head, qb, ki) assignment of the exp op to ACT or DVE.

    Returns dict (head, qb, ki) -> 'act' | 'dve'. Tiles containing
    q-columns < 256 must be exact (ACT); everything else goes to
    whichever engine has the lower modeled finish time.
    """
    ACT_NS_COL, ACT_NS_FIX = 1.01, 295.0
    DVE_NS_COL, DVE_NS_FIX = 1.04, 160.0
    # init with the out-copy work each engine does (split evenly):
    load = {"act": 0.0, "dve": 0.0}
    n_copies = HPC * NQB
    load["act"] += (n_copies / 2) * (QB * ACT_NS_COL + ACT_NS_FIX)
    load["dve"] += (n_copies / 2) * (QB * DVE_NS_COL + DVE_NS_FIX)
    plan = {}
    for head in range(HPC):
        for qb in range(NQB):
            q0 = QB * qb
            kmax = (q0 + QB) // 128
            for ki in range(kmax):
                sg = max(0, 128 * ki - q0)
                cols = QB - sg
                forced = q0 + sg < 256
                ca = cols * ACT_NS_COL + ACT_NS_FIX
                cd = cols * DVE_NS_COL + DVE_NS_FIX
                if forced or load["act"] + ca <= load["dve"] + cd:
                    plan[(head, qb, ki)] = "act"
                    load["act"] += ca
                else:
                    plan[(head, qb, ki)] = "dve"
                    load["dve"] += cd
    return plan


def _build_program():
    import concourse.bass as bass  # noqa: F401
    import concourse.mybir as mybir
    import concourse.tile as tile
    from concourse import bacc

    F16 = mybir.dt.float16
    F32 = mybir.dt.float32
    I16 = mybir.dt.int16
    EXP = mybir.ActivationFunctionType.Exp
    MULT = mybir.AluOpType.mult
    ADD = mybir.AluOpType.add

    plan = _plan_exp_engines()

    nc = bacc.Bacc()
    CMB = nc.declare_dram_parameter(
        "CMB", [128, NPAIR * PAIR_COLS], F16, isOutput=False
    )
    TRI = nc.declare_dram_parameter("TRI", [128, 640], F16, isOutput=False)
    OUT = nc.declare_dram_parameter("OUT", [HPC, D + 1, S], F16, isOutput=True)

    with tile.TileContext(nc) as tc:
        with (
            tc.tile_pool(name="cmbp", bufs=2) as cmbp,
            tc.tile_pool(name="singles", bufs=1) as singles,
            tc.tile_pool(name="etp", bufs=3) as etp,
            tc.tile_pool(name="obp", bufs=2) as obp,
            tc.tile_pool(name="stp", bufs=2, space="PSUM") as stp,
            tc.tile_pool(name="accp", bufs=2, space="PSUM") as accp,
        ):
            trib = singles.tile([128, 640], F16, tag="tri")
            nc.sync.dma_start(out=trib, in_=TRI[:])
            tri = trib[:, 0:128]

            # PE warm-up: the HAM clock gate only un-throttles (1.2 ->
            # 2.4 GHz) under sustained full-height (128-row) matmul
            # activity, and this kernel's 64-contraction score matmuls
            # never trigger it on their own. ~30 chained 512-col matmuls
            # warm the array while the first CMB DMA is in flight; once
            # warm, the dense matmul stream keeps it warm.
            for wi in range(30):
                wt = stp.tile([128, QB], F32, tag="st", name=f"warm{wi}")
                nc.tensor.matmul(
                    wt[:, 0:512], trib[:, 0:128], trib[:, 128:640],
                    start=True, stop=True,
                )

            copy_tick = 0
            for pair in range(NPAIR):
                cmb = cmbp.tile(
                    [128, PAIR_COLS], F16, tag="cmb", name=f"cmb{pair}"
                )
                nc.sync.dma_start(
                    out=cmb,
                    in_=CMB[:, pair * PAIR_COLS:(pair + 1) * PAIR_COLS],
                )
                for sub in range(2):
                    head = 2 * pair + sub
                    # kt holds both heads' K^T stacked ([0:64]=A, [64:128]=B);
                    # qt is this head's Q^T zero-padded on the other head's
                    # rows, so the full-height (128-row) matmul computes only
                    # this head's scores while keeping the PE activity monitor
                    # at full rate (a 64-row matmul stream lets the clock gate
                    # re-throttle to 1.2 GHz).
                    kt = cmb[:, 0:S]
                    qt = cmb[:, (1 + sub) * S:(2 + sub) * S]
                    va = cmb[:, 3 * S + sub * VCOLS: 3 * S + (sub + 1) * VCOLS]
                    va_r = va.rearrange("p (t c) -> p t c", t=NKT)

                    for qb in range(NQB):
                        q0 = QB * qb
                        kmax = (q0 + QB) // 128
                        acc = accp.tile(
                            [D + 1, QB], F32, tag="acc",
                            name=f"acc_h{head}_qb{qb}",
                        )
                        ets = {}

                        def do_st(ki):
                            """S^T matmuls for k-tile ki into a fresh PSUM
                            tile, then exp (+ diag mask) into SBUF fp16."""
                            sg = max(0, 128 * ki - q0)
                            st = stp.tile(
                                [128, QB], F32, tag="st",
                                name=f"st_h{head}_qb{qb}_k{ki}",
                            )
                            lhs_k = kt[:, 128 * ki:128 * (ki + 1)]
                            c0 = sg
                            while c0 < QB:
                                c1 = min(QB, (c0 // 512 + 1) * 512)
                                nc.tensor.matmul(
                                    st[:, c0:c1], lhs_k,
                                    qt[:, q0 + c0:q0 + c1],
                                    start=True, stop=True,
                                )
                                c0 = c1
                            et = etp.tile(
                                [128, QB], F16, tag="et",
                                name=f"et_h{head}_qb{qb}_k{ki}",
                            )
                            if plan[(head, qb, ki)] == "act":
                                nc.scalar.activation(
                                    et[:, sg:QB], st[:, sg:QB], EXP,
                                    scale=SCALE,
                                )
                            else:
                                nc.vector.tensor_scalar(
                                    et[:, sg:QB].bitcast(I16),
                                    st[:, sg:QB],
                                    FE_A, FE_B, MULT, ADD,
                                )
                            if 128 * ki >= q0:  # diagonal block in this tile
                                nc.gpsimd.tensor_tensor(
                                    et[:, sg:sg + 128],
                                    et[:, sg:sg + 128],
                                    tri[:, :],
                                    MULT,
                                )
                            ets[ki] = (et, sg)

                        def do_pv(ki):
                            et, sg = ets.pop(ki)
                            va_k = va_r[:, ki, :]  # [128, 65]
                            c0 = sg
                            while c0 < QB:
                                c1 = min(QB, (c0 // 512 + 1) * 512)
                                last_ki = (q0 + c1 - 1) // 128
                                nc.tensor.matmul(
                                    acc[:, c0:c1], va_k, et[:, c0:c1],
                                    start=(ki == 0),
                                    stop=(ki == last_ki),
                                )
                                c0 = c1

                        # software-pipelined issue: S(k0),S(k1),PV(k0),...
                        do_st(0)
                        for ki in range(1, kmax):
                            do_st(ki)
                            do_pv(ki - 1)
                        do_pv(kmax - 1)

                        ob = obp.tile(
                            [D + 1, QB], F16, tag="ob",
                            name=f"ob_h{head}_qb{qb}",
                        )
                        if copy_tick % 2 == 0:
                            nc.scalar.copy(ob, acc)
                        else:
                            nc.vector.tensor_copy(ob, acc)
                        copy_tick += 1
                        nc.sync.dma_start(
                            out=OUT[head, :, q0:q0 + QB], in_=ob,
                        )
    nc.finalize()
    return nc


def _get_program():
    if "nc" not in _prog_cache:
        _prog_cache["nc"] = _build_program()
    return _prog_cache["nc"]


def kernel(q, k, v, mask):
    global last_exec_time_ns
    q = np.asarray(q, dtype=np.float32)
    k = np.asarray(k, dtype=np.float32)
    v = np.asarray(v, dtype=np.float32)
    mask = np.asarray(mask).astype(bool)

    # This kernel specializes the causal (lower-triangular) mask from the
    # module; for any other mask fall back to a host reference.
    tril = np.tril(np.ones((S, S), dtype=bool))
    if mask.shape != (1, 1, S, S) or not np.array_equal(mask[0, 0], tril):
        scores = np.einsum("bhqd,bhkd->bhqk", q, k) / np.sqrt(np.float32(D))
        scores = np.where(mask, scores, -np.inf)
        m = scores.max(-1, keepdims=True)
        e = np.exp(scores - m)
        return (np.einsum("bhqk,bhkd->bhqd", e / e.sum(-1, keepdims=True), v)
                .astype(np.float32))

    _install_trace_hook()
    from concourse.bass_utils import run_bass_kernel_spmd

    nc = _get_program()

    qf = q.reshape(B * H, S, D).astype(np.float16)
    kf = k.reshape(B * H, S, D).astype(np.float16)
    vf = v.reshape(B * H, S, D).astype(np.float16)

    tri_np = np.ones((128, 640), dtype=np.float16)
    tri_np[:, 0:128] = np.triu(np.ones((128, 128), dtype=np.float16))  # k<=q

    in_maps = []
    for core in range(NCORES):
        pairs = []
        for p in range(NPAIR):
            hA = core * HPC + 2 * p
            hB = hA + 1
            ktp = np.concatenate(
                [kf[hA].T, kf[hB].T], axis=0
            )  # [128, 2048]
            z = np.zeros((64, S), dtype=np.float16)
            qta = np.concatenate([qf[hA].T, z], axis=0)
            qtb = np.concatenate([z, qf[hB].T], axis=0)
            vas = []
            for h in (hA, hB):
                vt = vf[h].reshape(NKT, 128, D).transpose(1, 0, 2)
                va = np.concatenate(
                    [vt, np.ones((128, NKT, 1), dtype=np.float16)], axis=2
                ).reshape(128, VCOLS)
                vas.append(va)
            pairs.append(
                np.concatenate([ktp, qta, qtb, vas[0], vas[1]], axis=1)
            )
        cmb = np.ascontiguousarray(np.concatenate(pairs, axis=1))
        in_maps.append({"CMB": cmb, "TRI": tri_np})

    trace = bool(os.environ.get("ATTN_TRACE"))
    res = run_bass_kernel_spmd(
        nc, in_maps, list(range(NCORES)), trace=trace
    )
    last_exec_time_ns = res.exec_time_ns

    out = np.empty((B * H, S, D), dtype=np.float32)
    for core in range(NCORES):
        acc = res.results[core]["OUT"].astype(np.float32)  # [HPC, 65, S]
        o = acc[:, :D, :] / acc[:, D:D + 1, :]
        out[core * HPC:(core + 1) * HPC] = o.transpose(0, 2, 1)
    return out.reshape(B, H, S, D)


# revision 13
# speedup vs baseline: 2.7112x; 1.3018x over previous
"""Causal attention (B=4,H=16,S=2048,D=64) on 8 NeuronCores via Bass/Tile.

v2 strategy (per core = 8 heads of the 64 B*H heads):
- All matmul inputs in fp16 (1 cyc/col on the PE array vs ~3-4 for fp32r).
- Layout per head: K^T/Q^T [64, S] fp16; V augmented with a ones column
  (denominator) as [128, 16, 65] fp16.
- Loop: 2 q-blocks of 1024 per head; per (qb, ki) a [128, <=1024] score
  tile S^T = K_tile.T @ Q in PSUM; exp is computed either exactly on the
  Scalar engine (ACT, with scale=1/8 folded) or via a one-instruction
  Schraudolph fast-exp on the Vector engine (tensor_scalar writing the
  fp16 bit pattern through an int16 output); tiles covering q<256 are
  forced to the exact path, the rest are greedily load-balanced.
- The causal triangle on diagonal 128-blocks is applied by multiplying
  the exp tile with a 0/1 triangle on GpSimd (no -inf mask matmuls).
- PV: acc[65, 1024] += V_aug.T @ E per k-tile, accumulated in PSUM.
- Accumulators are copied to SBUF fp16 (alternating ACT/DVE) and DMA'd
  out; host divides by the denominator row and transposes back.
- No max-subtraction: scores*scale ~ N(0,1), exp stays in range.
"""
import os
import sys

sys.path.insert(0, "/opt/trn_rl_repo")

import numpy as np

B, H, S, D = 4, 16, 2048, 64
NCORES = 8
HPC = (B * H) // NCORES        # heads per core = 8
NKT = S // 128                 # 128-wide k-tiles per head = 16
QB = 1024                      # q-block width
NQB = S // QB                  # q blocks per head = 2
VCOLS = NKT * (D + 1)          # 16*65 = 1040
PAIR_COLS = 3 * S + 2 * VCOLS  # KT | QTA | QTB | V_a | V_b = 8224
NPAIR = HPC // 2               # 4
SCALE = 1.0 / 8.0              # 1/sqrt(D)

# Schraudolph fast-exp onto the fp16 bit pattern:
#   i16 = round(raw_score * FE_A + FE_B);  bitcast(i16) ~= exp(raw*SCALE)
# max rel err ~3.0%; only used for tiles whose q-columns are all >= 256
# (rows there average >=257 softmax terms, so the sawtooth error cancels).
FE_A = float(1024.0 * np.log2(np.e) * SCALE)
FE_B = float(15360.0 - 44.7)

last_exec_time_ns = None

_prog_cache = {}


def _install_trace_hook():
    """Inject antenv.axon_hooks (missing from this image) so trace=True can
    capture NTFF profiles. Degrades silently if anything is unavailable."""
    import types

    try:
        import antenv

        if "antenv.axon_hooks" in sys.modules:
            return
        mod = types.ModuleType("antenv.axon_hooks")
        state = {"hook": None}
        mod.set_axon_ntff_profile_hook = lambda h: state.__setitem__("hook", h)
        mod.get_axon_ntff_profile_hook = lambda: state["hook"]
        sys.modules["antenv.axon_hooks"] = mod
        antenv.axon_hooks = mod
        from trn_agent_boot.trn_boot import _ntff_profile_via_ctypes

        hook = _ntff_profile_via_ctypes("/opt/axon/libaxon_pjrt.so")
        if hook is not None:
            mod.set_axon_ntff_profile_hook(hook)
    except Exception:
        pass


def _plan_exp_engines():
    """Static per-(head, qb, ki) assignment of the exp op to ACT or DVE.

    Returns dict (head, qb, ki) -> 'act' | 'dve'. Tiles containing
    q-columns < 256 must be exact (ACT); everything else goes to
    whichever engine has the lower modeled finish time.
    """
    ACT_NS_COL, ACT_NS_FIX = 1.01, 295.0
    DVE_NS_COL, DVE_NS_FIX = 1.04, 160.0
    # init with the out-copy work each engine does (split evenly):
    load = {"act": 0.0, "dve": 0.0}
    n_copies = HPC * NQB
    load["act"] += (n_copies / 2) * (QB * ACT_NS_COL + ACT_NS_FIX)
    load["dve"] += (n_copies / 2) * (QB * DVE_NS_COL + DVE_NS_FIX)
    plan = {}
    for head in range(HPC):
        for qb in range(NQB):
            q0 = QB * qb
            kmax = (q0 + QB) // 128
            for ki in range(kmax):
                sg = max(0, 128 * ki - q0)
                cols = QB - sg
                forced = q0 + sg < 256
                ca = cols * ACT_NS_COL + ACT_NS_FIX
                cd = cols * DVE_NS_COL + DVE_NS_FIX
                if forced or load["act"] + ca <= load["dve"] + cd:
                    plan[(head, qb, ki)] = "act"
                    load["act"] += ca
                else:
                    plan[(head, qb, ki)] = "dve"
                    load["dve"] += cd
    return plan


def _build_program():
    import concourse.bass as bass  # noqa: F401
    import concourse.mybir as mybir
    import concourse.tile as tile
    from concourse import bacc

    F16 = mybir.dt.float16
    F32 = mybir.dt.float32
    I16 = mybir.dt.int16
    EXP = mybir.ActivationFunctionType.Exp
    MULT = mybir.AluOpType.mult
    ADD = mybir.AluOpType.add

    plan = _plan_exp_engines()

    nc = bacc.Bacc()
    CMB = nc.declare_dram_parameter(
        "CMB", [128, NPAIR * PAIR_COLS], F16, isOutput=False
    )
    TRI = nc.declare_dram_parameter("TRI", [128, 640], F16, isOutput=False)
    OUT = nc.declare_dram_parameter("OUT", [HPC, D + 1, S], F16, isOutput=True)

    with tile.TileContext(nc) as tc:
        with (
            tc.tile_pool(name="cmbp", bufs=2) as cmbp,
            tc.tile_pool(name="singles", bufs=1) as singles,
            tc.tile_pool(name="etp", bufs=4) as etp,
            tc.tile_pool(name="obp", bufs=2) as obp,
            tc.tile_pool(name="stp", bufs=3, space="PSUM") as stp,
            tc.tile_pool(name="accp", bufs=1, space="PSUM") as accp,
        ):
            trib = singles.tile([128, 640], F16, tag="tri")
            nc.sync.dma_start(out=trib, in_=TRI[:])
            # causal mask as a rank-128 matmul: mska.T @ mskb adds -60000
            # to diagonal-block entries with k_rel > q_rel; exp maps them
            # to 0 (the DVE fast-exp saturates its int16 convert to
            # -32768 = fp16 -0.0).
            mska = trib[:, 0:128]
            mskb = trib[:, 128:256]

            # PE warm-up: the HAM clock gate only un-throttles (1.2 ->
            # 2.4 GHz) under sustained full-height (128-row) matmul
            # activity, and this kernel's 64-contraction score matmuls
            # never trigger it on their own. ~30 chained 512-col matmuls
            # warm the array while the first CMB DMA is in flight; once
            # warm, the dense matmul stream keeps it warm.
            for wi in range(30):
                wt = stp.tile([128, QB], F32, tag="st", name=f"warm{wi}")
                nc.tensor.matmul(
                    wt[:, 0:512], trib[:, 0:128], trib[:, 128:640],
                    start=True, stop=True,
                )

            copy_tick = 0
            for pair in range(NPAIR):
                cmb = cmbp.tile(
                    [128, PAIR_COLS], F16, tag="cmb", name=f"cmb{pair}"
                )
                nc.sync.dma_start(
                    out=cmb,
                    in_=CMB[:, pair * PAIR_COLS:(pair + 1) * PAIR_COLS],
                )
                for sub in range(2):
                    head = 2 * pair + sub
                    # kt holds both heads' K^T stacked ([0:64]=A, [64:128]=B);
                    # qt is this head's Q^T zero-padded on the other head's
                    # rows, so the full-height (128-row) matmul computes only
                    # this head's scores while keeping the PE activity monitor
                    # at full rate (a 64-row matmul stream lets the clock gate
                    # re-throttle to 1.2 GHz).
                    kt = cmb[:, 0:S]
                    qt = cmb[:, (1 + sub) * S:(2 + sub) * S]
                    va = cmb[:, 3 * S + sub * VCOLS: 3 * S + (sub + 1) * VCOLS]
                    va_r = va.rearrange("p (t c) -> p t c", t=NKT)

                    for qb in range(NQB):
                        q0 = QB * qb
                        kmax = (q0 + QB) // 128
                        acc = accp.tile(
                            [D + 1, QB], F32, tag="acc",
                            name=f"acc_h{head}_qb{qb}",
                        )
                        ets = {}

                        def do_st(ki):
                            """S^T matmuls for k-tile ki into a fresh PSUM
                            tile (causal -60000 mask folded in as an extra
                            PE matmul on the diagonal block), then exp into
                            SBUF fp16."""
                            sg = max(0, 128 * ki - q0)
                            st = stp.tile(
                                [128, QB], F32, tag="st",
                                name=f"st_h{head}_qb{qb}_k{ki}",
                            )
                            lhs_k = kt[:, 128 * ki:128 * (ki + 1)]
                            c0 = sg
                            while c0 < QB:
                                c1 = min(QB, (c0 // 512 + 1) * 512)
                                nc.tensor.matmul(
                                    st[:, c0:c1], lhs_k,
                                    qt[:, q0 + c0:q0 + c1],
                                    start=True, stop=True,
                                )
                                c0 = c1
                            if 128 * ki >= q0:  # diagonal block in this tile
                                nc.tensor.matmul(
                                    st[:, sg:sg + 128], mska, mskb,
                                    start=False, stop=True,
                                )
                            et = etp.tile(
                                [128, QB], F16, tag="et",
                                name=f"et_h{head}_qb{qb}_k{ki}",
                            )
                            if plan[(head, qb, ki)] == "act":
                                nc.scalar.activation(
                                    et[:, sg:QB], st[:, sg:QB], EXP,
                                    scale=SCALE,
                                )
                            else:
                                nc.vector.tensor_scalar(
                                    et[:, sg:QB].bitcast(I16),
                                    st[:, sg:QB],
                                    FE_A, FE_B, MULT, ADD,
                                )
                            ets[ki] = (et, sg)

                        def do_pv(ki):
                            et, sg = ets.pop(ki)
                            va_k = va_r[:, ki, :]  # [128, 65]
                            c0 = sg
                            while c0 < QB:
                                c1 = min(QB, (c0 // 512 + 1) * 512)
                                last_ki = (q0 + c1 - 1) // 128
                                nc.tensor.matmul(
                                    acc[:, c0:c1], va_k, et[:, c0:c1],
                                    start=(ki == 0),
                                    stop=(ki == last_ki),
                                )
                                c0 = c1

                        # software-pipelined issue, depth 2:
                        # S(k0),S(k1),S(k2),PV(k0),S(k3),PV(k1),...
                        do_st(0)
                        do_st(1)
                        for ki in range(2, kmax):
                            do_st(ki)
                            do_pv(ki - 2)
                        do_pv(kmax - 2)
                        do_pv(kmax - 1)

                        # copy-out split across both engines to shorten the
                        # block-boundary stall (accp is single-buffered)
                        ob = obp.tile(
                            [D + 1, QB], F16, tag="ob",
                            name=f"ob_h{head}_qb{qb}",
                        )
                        nc.scalar.copy(ob[:, 0:512], acc[:, 0:512])
                        nc.vector.tensor_copy(ob[:, 512:QB], acc[:, 512:QB])
                        copy_tick += 1
                        nc.sync.dma_start(
                            out=OUT[head, :, q0:q0 + QB], in_=ob,
                        )
    nc.finalize()
    return nc


def _get_program():
    if "nc" not in _prog_cache:
        _prog_cache["nc"] = _build_program()
    return _prog_cache["nc"]


def kernel(q, k, v, mask):
    global last_exec_time_ns
    q = np.asarray(q, dtype=np.float32)
    k = np.asarray(k, dtype=np.float32)
    v = np.asarray(v, dtype=np.float32)
    mask = np.asarray(mask).astype(bool)

    # This kernel specializes the causal (lower-triangular) mask from the
    # module; for any other mask fall back to a host reference.
    tril = np.tril(np.ones((S, S), dtype=bool))
    if mask.shape != (1, 1, S, S) or not np.array_equal(mask[0, 0], tril):
        scores = np.einsum("bhqd,bhkd->bhqk", q, k) / np.sqrt(np.float32(D))
        scores = np.where(mask, scores, -np.inf)
        m = scores.max(-1, keepdims=True)
        e = np.exp(scores - m)
        return (np.einsum("bhqk,bhkd->bhqd", e / e.sum(-1, keepdims=True), v)
                .astype(np.float32))

    _install_trace_hook()
    from concourse.bass_utils import run_bass_kernel_spmd

    nc = _get_program()

    qf = q.reshape(B * H, S, D).astype(np.float16)
    kf = k.reshape(B * H, S, D).astype(np.float16)
    vf = v.reshape(B * H, S, D).astype(np.float16)

    tri_np = np.ones((128, 640), dtype=np.float16)
    # mska[j, r] = 1 for r > j (row 127 zero); mskb = diag(-60000)
    A = (np.arange(128)[None, :] > np.arange(128)[:, None]).astype(np.float16)
    A[127, :] = 0
    Bm = np.zeros((128, 128), dtype=np.float16)
    idx = np.arange(127)
    Bm[idx, idx] = np.float16(-60000.0)
    tri_np[:, 0:128] = A
    tri_np[:, 128:256] = Bm

    in_maps = []
    for core in range(NCORES):
        pairs = []
        for p in range(NPAIR):
            hA = core * HPC + 2 * p
            hB = hA + 1
            ktp = np.concatenate(
                [kf[hA].T, kf[hB].T], axis=0
            )  # [128, 2048]
            z = np.zeros((64, S), dtype=np.float16)
            qta = np.concatenate([qf[hA].T, z], axis=0)
            qtb = np.concatenate([z, qf[hB].T], axis=0)
            vas = []
            for h in (hA, hB):
                vt = vf[h].reshape(NKT, 128, D).transpose(1, 0, 2)
                va = np.concatenate(
                    [vt, np.ones((128, NKT, 1), dtype=np.float16)], axis=2
                ).reshape(128, VCOLS)
                vas.append(va)
            pairs.append(
                np.concatenate([ktp, qta, qtb, vas[0], vas[1]], axis=1)
            )
        cmb = np.ascontiguousarray(np.concatenate(pairs, axis=1))
        in_maps.append({"CMB": cmb, "TRI": tri_np})

    trace = bool(os.environ.get("ATTN_TRACE"))
    res = run_bass_kernel_spmd(
        nc, in_maps, list(range(NCORES)), trace=trace
    )
    last_exec_time_ns = res.exec_time_ns

    out = np.empty((B * H, S, D), dtype=np.float32)
    for core in range(NCORES):
        acc = res.results[core]["OUT"].astype(np.float32)  # [HPC, 65, S]
        o = acc[:, :D, :] / acc[:, D:D + 1, :]
        out[core * HPC:(core + 1) * HPC] = o.transpose(0, 2, 1)
    return out.reshape(B, H, S, D)
